# revision 40
# baseline (speedup 1.0000x reference)
"""GCF message passing on 8 trn2 cores — merged-cell windowed-SpMM.

Per core (dest-shard of 12500 nodes), per layer:
  SpMM: edges sorted by (group, src-range, dest); chunks of 128 edge
        slots packed contiguously per (group, range) cell — a chunk may
        span window boundaries; each window's matmul selects its own O
        columns (zero rows for foreign slots). O is uint8 (val*2550),
        streamed via the ACT HWDGE queue and cast to fp16 on DVE; the
        1/2550 dequant rides the PSUM->SBUF copy scale. Gathered source
        rows (fp16, 256B) come from HBM dma_gather with exact
        num_idxs_reg (trailing -1 idx trimmed by the Q7 ucode) — the
        kernel is SWDGE-generation-bound at ~2.5ns/idx aggregate.
  Dense: y^T = Wlin@(Lx+F)^T + Wint@(Lx*F)^T per 512-block, fused
        bias+leaky-relu on ACT, row norm via ones-matmul +
        Abs_reciprocal_sqrt (ACT) + broadcast-matmul.
  Share: PE-transpose shard -> fshard (ping-pong) -> AllGather ags[l];
        final-dot level l is computed before collective l fires.
Final: per concat level gather u/i rows, multiply + reduce, accumulate.
"""

import os

import numpy as np

import concourse.bacc as bacc
import concourse.mybir as mybir
import concourse.tile as tile
from concourse.bass_utils import run_bass_kernel_spmd
from concourse.masks import make_identity

NUM_USERS = 30000
NUM_ITEMS = 70000
N = 100000
D = 128
NL = 3
BATCH = 16384
NCORE = 8
SHARD = N // NCORE
RS = 25000
NR = 4
WCAP = 128
KMAX = 3
GM = 48                # target chunks per gather group
VSCALE = 2550.0        # uint8 quantization scale for edge vals
BSH = BATCH // NCORE   # 2048
EPS = 1e-12
SLOPE = 0.01

f32 = mybir.dt.float32
f16 = mybir.dt.float16
i16 = mybir.dt.int16
u8 = mybir.dt.uint8

NQUEUES = int(os.environ.get("KQ", "4"))

_cache = {}


# ---------------------------------------------------------------- host side
def _build_structure(rows, cols):
    core = rows // SHARD
    dloc = rows - core * SHARD
    rng = cols // RS

    counts = np.zeros((NCORE, SHARD, NR), np.int32)
    np.add.at(counts, (core, dloc, rng), 1)

    windows = []
    cum = np.zeros((NCORE, NR), np.int64)
    d0 = 0
    for d in range(SHARD):
        c = counts[:, d, :]
        if (cum + c > 128 * KMAX).any() or d - d0 >= WCAP:
            windows.append((d0, d - d0))
            d0 = d
            cum = c.astype(np.int64).copy()
        else:
            cum += c
    windows.append((d0, SHARD - d0))
    nwin = len(windows)

    # per (window, range, core) edge counts
    wsum = np.zeros((nwin, NR, NCORE), np.int64)
    for i, (a, w) in enumerate(windows):
        wsum[i] = counts[:, a:a + w, :].sum(axis=1).T

    # group formation on estimated chunk counts
    est = np.ceil(wsum.max(axis=2) / 128).astype(np.int64).sum(axis=1)
    groups = []
    gstart, cnt = 0, 0
    for i in range(nwin):
        c = int(est[i])
        if cnt + c > GM and cnt > 0:
            groups.append((gstart, i))
            gstart, cnt = i, 0
        cnt += c
    groups.append((gstart, nwin))
    NG = len(groups)

    # merged cells: chunks packed contiguously per (group, range);
    # a chunk may span window boundaries. Shared (max-over-core) sizing.
    group_info = []     # (ncols, rlist, (wa, wb)); rlist=(r,cbase,nch,nidx)
    mm_lists = []       # per group: {win: [(ci_local, oloc)]}, ordered
    o_off_cursor = 0
    grp_obase, grp_osize = [], []
    chunk_base = [0]
    omap = {}           # (win, global_chunk) -> oloc
    for gi, (wa, wb) in enumerate(groups):
        col = 0
        rlist = []
        mm = {i: [] for i in range(wa, wb)}
        gob = o_off_cursor
        for r in range(NR):
            cellcnt = wsum[wa:wb, r, :].sum(axis=0)   # per core
            mx = int(cellcnt.max())
            if mx == 0:
                continue
            nidx = -(-mx // 16) * 16
            nch = -(-nidx // 128)
            # per-core prefix of window starts within the cell
            pre = np.zeros((wb - wa + 1, NCORE), np.int64)
            np.cumsum(wsum[wa:wb, r, :], axis=0, out=pre[1:])
            for iw in range(wa, wb):
                s0 = pre[iw - wa]
                s1 = pre[iw - wa + 1]
                act = s1 > s0
                if not act.any():
                    continue
                c_lo = int((s0[act] // 128).min())
                c_hi = int(((s1[act] - 1) // 128).max())
                for cl in range(c_lo, c_hi + 1):
                    mm[iw].append((col + cl, r))
            rlist.append((r, col, nch, nidx))
            col += nch
        # O layout + guard: every window needs >= 1 mm entry
        mm_seq = {}
        for iw in range(wa, wb):
            if not mm[iw]:
                mm[iw].append((0, -1))
            ents = []
            w = windows[iw][1]
            for (cl, r) in mm[iw]:
                oloc = o_off_cursor - gob
                omap[(iw, chunk_base[-1] + cl)] = o_off_cursor
                o_off_cursor += w
                ents.append((cl, oloc))
            mm_seq[iw] = tuple(ents)
        group_info.append((col, tuple(rlist), (wa, wb)))
        mm_lists.append(mm_seq)
        grp_obase.append(gob)
        grp_osize.append(o_off_cursor - gob)
        chunk_base.append(chunk_base[-1] + col)

    NCH = chunk_base[-1]
    g_of_win = np.zeros(nwin, np.int64)
    for gi, (wa, wb) in enumerate(groups):
        g_of_win[wa:wb] = gi
    return dict(windows=tuple(windows), groups=tuple(groups),
                group_info=tuple(group_info),
                mm_lists=tuple(mm_lists),
                grp_obase=tuple(grp_obase), grp_osize=tuple(grp_osize),
                chunk_base=tuple(chunk_base), g_of_win=g_of_win,
                omap=omap, OW=int(o_off_cursor), NCH=NCH)


def _pack_edges(rows, cols, vals, st):
    core = rows // SHARD
    dloc = rows - core * SHARD
    rng = cols // RS
    windows = st["windows"]
    nwin = len(windows)
    NCH = st["NCH"]
    NG = len(st["groups"])

    win_of = np.zeros(SHARD, np.int64)
    d0s = np.zeros(nwin, np.int64)
    for i, (a, w) in enumerate(windows):
        win_of[a:a + w] = i
        d0s[i] = a
    win_e = win_of[dloc]
    g_e = st["g_of_win"][win_e]

    # sort edges by (core, group, range, dest) -> contiguous cell streams
    key = (((core * NG + g_e) * NR + rng) * SHARD + dloc)
    order = np.argsort(key, kind="stable")
    cellkey = (core[order] * NG + g_e[order]) * NR + rng[order]
    uniq, inv, cnt = np.unique(cellkey, return_inverse=True,
                               return_counts=True)
    starts = np.zeros(len(uniq), np.int64)
    np.cumsum(cnt[:-1], out=starts[1:])
    pos = np.arange(len(cellkey)) - starts[inv]

    core_s = core[order]
    g_s = g_e[order]
    r_s = rng[order]
    win_s = win_e[order]

    # cell cbase lookup: (g, r) -> cbase
    cb = np.full((NG, NR), -1, np.int64)
    for gi, (ncols, rlist, _) in enumerate(st["group_info"]):
        for (r, cbase, nch, nidx) in rlist:
            cb[gi, r] = cbase
    ci_e = np.asarray(st["chunk_base"])[g_s] + cb[g_s, r_s] + pos // 128
    p_e = pos % 128

    eidx = np.zeros((NCORE, 16, NCH * 8), np.int16)
    col_local = (cols[order] - r_s * RS).astype(np.int16)
    eidx[core_s, p_e % 16, ci_e * 8 + p_e // 16] = col_local
    # shared trailing trim: mark [nidx, nch*128) positions as -1
    for gi, (ncols, rlist, _) in enumerate(st["group_info"]):
        for (r, cbase, nch, nidx) in rlist:
            for q in range(nidx, nch * 128):
                ci = st["chunk_base"][gi] + cbase + q // 128
                eidx[:, q % 16, ci * 8 + (q % 128) // 16] = -1

    # O fill via omap lookup table
    omap_arr = np.full((nwin, NCH), -1, np.int64)
    for (iw, ci), oloc in st["omap"].items():
        omap_arr[iw, ci] = oloc
    obase_e = omap_arr[win_s, ci_e]
    assert (obase_e >= 0).all()
    O = np.zeros((NCORE, 128, st["OW"]), np.uint8)
    j_e = dloc[order] - d0s[win_s]
    q = np.clip(np.round(vals[order] * VSCALE), 0, 255).astype(np.uint8)
    O[core_s, p_e, obase_e + j_e] = q
    return eidx, O


def _pack_final(userIdx, itemIdx):
    irow = itemIdx + NUM_USERS
    ir = irow // RS
    nfb_counts = np.zeros((NCORE, NR), np.int64)
    perms = []
    for c in range(NCORE):
        sl = slice(c * BSH, (c + 1) * BSH)
        o = np.argsort(ir[sl], kind="stable")
        perms.append(o)
        nfb_counts[c] = np.bincount(ir[sl][o], minlength=NR)
    bucket_chunks = np.ceil(nfb_counts.max(axis=0) / 128).astype(np.int64)
    fin_bounds = []
    c0 = 0
    for r in range(NR):
        n = int(bucket_chunks[r])
        if n == 0:
            continue
        fin_bounds.append((r, c0, n))
        c0 += n
    NFB = c0

    uidx_arr = np.zeros((NCORE, 16, NFB * 8), np.int16)
    iidx_arr = np.zeros((NCORE, 16, NFB * 8), np.int16)
    inv_perm = np.full((NCORE, NFB * 128), -1, np.int64)
    for c in range(NCORE):
        sl = slice(c * BSH, (c + 1) * BSH)
        o = perms[c]
        u_s = userIdx[sl][o]
        i_s = irow[sl][o]
        r_s = ir[sl][o]
        jpos = np.zeros(BSH, np.int64)
        for (r, b0, nchk) in fin_bounds:
            m = r_s == r
            jpos[m] = b0 * 128 + np.arange(int(m.sum()))
        uidx_arr[c, jpos % 16, (jpos // 128) * 8 + (jpos % 128) // 16] = \
            u_s.astype(np.int16)
        iidx_arr[c, jpos % 16, (jpos // 128) * 8 + (jpos % 128) // 16] = \
            (i_s - r_s * RS).astype(np.int16)
        inv_perm[c, jpos] = np.arange(c * BSH, (c + 1) * BSH)[o]
    return NFB, tuple(fin_bounds), uidx_arr, iidx_arr, inv_perm


def _pack_inputs(userIdx, itemIdx, rows, cols, vals, uEmbd, iEmbd,
                 Wlin, blin, Wint, bint):
    rows = np.asarray(rows, dtype=np.int64)
    cols = np.asarray(cols, dtype=np.int64)
    vals = np.asarray(vals, dtype=np.float32)
    userIdx = np.asarray(userIdx, dtype=np.int64)
    itemIdx = np.asarray(itemIdx, dtype=np.int64)

    feat0 = np.concatenate([np.asarray(uEmbd, np.float32),
                            np.asarray(iEmbd, np.float32)], axis=0)
    feat16 = np.ascontiguousarray(feat0.astype(np.float16))

    st = _build_structure(rows, cols)
    eidx, O = _pack_edges(rows, cols, vals, st)
    NFB, fin_bounds, uidx_arr, iidx_arr, inv_perm = _pack_final(
        userIdx, itemIdx)

    wlin_h = np.ascontiguousarray(
        np.asarray(Wlin, np.float32).transpose(1, 0, 2).reshape(D, NL * D)
    ).astype(np.float16)
    wint_h = np.ascontiguousarray(
        np.asarray(Wint, np.float32).transpose(1, 0, 2).reshape(D, NL * D)
    ).astype(np.float16)
    biasc = np.ascontiguousarray(
        (np.asarray(blin, np.float32) + np.asarray(bint, np.float32)).T)

    in_maps = []
    for c in range(NCORE):
        f0t = np.ascontiguousarray(
            feat16[c * SHARD:(c + 1) * SHARD].T)
        in_maps.append({
            "feat16": feat16,
            "f0t": f0t,
            "eidx": np.ascontiguousarray(np.tile(eidx[c], (8, 1))),
            "odram": np.ascontiguousarray(O[c]),
            "wlin": wlin_h,
            "wint": wint_h,
            "biasc": biasc,
            "uidx": np.ascontiguousarray(np.tile(uidx_arr[c], (8, 1))),
            "iidx": np.ascontiguousarray(np.tile(iidx_arr[c], (8, 1))),
        })

    meta = (st, NFB, fin_bounds)
    return meta, in_maps, inv_perm


# ---------------------------------------------------------------- kernel
def _build(meta):
    st, NFB, fin_bounds = meta
    windows = st["windows"]
    group_info = st["group_info"]
    mm_lists = st["mm_lists"]
    chunk_base = st["chunk_base"]
    grp_obase = st["grp_obase"]
    grp_osize = st["grp_osize"]
    NCH = st["NCH"]
    OW = st["OW"]
    nwin = len(windows)

    maxgc = max(g[0] for g in group_info)
    maxgo = max(grp_osize)

    nc = bacc.Bacc(num_devices=NCORE, num_swdge_queues=max(NQUEUES, 1))

    feat16 = nc.dram_tensor("feat16", [N, D], f16, kind="ExternalInput")
    f0t = nc.dram_tensor("f0t", [D, SHARD], f16, kind="ExternalInput")
    eidx = nc.dram_tensor("eidx", [128, NCH * 8], i16, kind="ExternalInput")
    odram = nc.dram_tensor("odram", [128, OW], u8, kind="ExternalInput")
    wlin = nc.dram_tensor("wlin", [D, NL * D], f16, kind="ExternalInput")
    wint = nc.dram_tensor("wint", [D, NL * D], f16, kind="ExternalInput")
    biasc = nc.dram_tensor("biasc", [D, NL], f32, kind="ExternalInput")
    uidx = nc.dram_tensor("uidx", [128, NFB * 8], i16, kind="ExternalInput")
    iidx = nc.dram_tensor("iidx", [128, NFB * 8], i16, kind="ExternalInput")
    score = nc.dram_tensor("score", [128, NFB], f32, kind="ExternalOutput")

    add = mybir.AluOpType.add
    mult = mybir.AluOpType.mult
    maxop = mybir.AluOpType.max
    AF = mybir.ActivationFunctionType

    with tile.TileContext(nc) as tc:
        with (
            tc.tile_pool(name="const", bufs=1) as cp,
            tc.tile_pool(name="ft", bufs=1) as ftp,
            tc.tile_pool(name="g", bufs=4) as gp,
            tc.tile_pool(name="ob8", bufs=2) as op8,
            tc.tile_pool(name="ob", bufs=2) as op_,
            tc.tile_pool(name="sb", bufs=3) as sbp,
            tc.tile_pool(name="fin", bufs=2) as fp_,
            tc.tile_pool(name="pw", bufs=4, space="PSUM") as pwp,
            tc.tile_pool(name="py", bufs=2, space="PSUM") as pyp,
            tc.tile_pool(name="pn", bufs=1, space="PSUM") as pnp,
            tc.tile_pool(name="pbc", bufs=1, space="PSUM") as pbc,
            tc.tile_pool(name="dram", bufs=1, space="DRAM") as dp,
        ):
            # ---------- constants ----------
            ident = cp.tile([128, 128], f16)
            make_identity(nc, ident[:])
            ones_c = cp.tile([128, 1], f16)
            nc.vector.memset(ones_c[:], 1.0)
            onesrow = cp.tile([1, 128], f16)
            nc.vector.memset(onesrow[:], 1.0)

            wlin_sb = cp.tile([128, NL * 128], f16)
            nc.sync.dma_start(out=wlin_sb[:], in_=wlin[:])
            wint_sb = cp.tile([128, NL * 128], f16)
            nc.sync.dma_start(out=wint_sb[:], in_=wint[:])
            bias_sb = cp.tile([128, NL], f32)
            nc.sync.dma_start(out=bias_sb[:], in_=biasc[:])

            eidx_sb = cp.tile([128, NCH * 8], i16)
            nc.sync.dma_start(out=eidx_sb[:], in_=eidx[:])
            uidx_sb = cp.tile([128, NFB * 8], i16)
            nc.sync.dma_start(out=uidx_sb[:], in_=uidx[:])
            iidx_sb = cp.tile([128, NFB * 8], i16)
            nc.sync.dma_start(out=iidx_sb[:], in_=iidx[:])

            for ii in range(4):
                gtz = gp.tile([128, maxgc * 128], f16, tag="gt",
                              name=f"gtinit{ii}")
                nc.vector.memset(gtz[:], 0.0)

            FTP = 12544  # SHARD padded to x128 for PE transposes
            fta = ftp.tile([128, FTP], f16, tag="fta")
            ftb = ftp.tile([128, FTP], f16, tag="ftb")
            nc.vector.memset(fta[:, SHARD:], 0.0)
            nc.vector.memset(ftb[:, SHARD:], 0.0)
            nc.sync.dma_start(out=fta[:, :SHARD], in_=f0t[:])

            fshards = [dp.tile([SHARD, D], f16, name=f"fsh{i}", tag=f"fsh{i}")
                       for i in range(2)]
            ags = [dp.tile([N, D], f16, name=f"ag{i}", tag=f"ag{i}",
                           addr_space="Shared") for i in range(NL)]

            acc_t = cp.tile([128, NFB], f32)

            def emit_final_level(li, srcf):
                ug = fp_.tile([128, NFB * 128], f16, tag="ug",
                              name=f"ug{li}")
                nc.gpsimd.dma_gather(
                    ug[:].rearrange("p (c d) -> p c d", d=128),
                    srcf[:],
                    uidx_sb[:],
                    NFB * 128, NFB * 128, 128,
                    single_packet=False, queue_num=(2 * li) % NQUEUES,
                )
                ig = fp_.tile([128, NFB * 128], f16, tag="ig",
                              name=f"ig{li}")
                for (rr, c0, cn) in fin_bounds:
                    nc.gpsimd.dma_gather(
                        ig[:, c0 * 128:(c0 + cn) * 128].rearrange(
                            "p (c d) -> p c d", d=128),
                        srcf[rr * RS:, :],
                        iidx_sb[:, c0 * 8:(c0 + cn) * 8],
                        cn * 128, cn * 128, 128,
                        single_packet=False,
                        queue_num=(2 * li + 1 + rr) % NQUEUES,
                    )
                nc.vector.tensor_tensor(out=ug[:], in0=ug[:], in1=ig[:],
                                        op=mult)
                sc = sbp.tile([128, NFB], f32, tag="sc")
                nc.vector.tensor_reduce(
                    out=sc[:],
                    in_=ug[:].rearrange("p (c d) -> p c d", d=128),
                    axis=mybir.AxisListType.X, op=add)
                if li == 0:
                    nc.vector.tensor_copy(acc_t[:], sc[:])
                else:
                    nc.vector.tensor_tensor(out=acc_t[:], in0=acc_t[:],
                                            in1=sc[:], op=add)

            for l in range(NL):
                fshard = fshards[l % 2]
                ftin = fta if l % 2 == 0 else ftb
                ftout = ftb if l % 2 == 0 else fta
                src = feat16 if l == 0 else ags[l - 1]

                # ---------- SpMM ----------
                blocks_done = 0
                for gi, (ncols, rlist, (wa, wb)) in enumerate(group_info):
                    gt = gp.tile([128, maxgc * 128], f16, tag="gt",
                                 name=f"gt{l}_{gi}")
                    for (r, cbase, nch, nidx) in rlist:
                        q = (r % NQUEUES) if NQUEUES > 1 else 0
                        nc.gpsimd.dma_gather(
                            gt[:, cbase * 128:(cbase + nch) * 128].rearrange(
                                "p (c d) -> p c d", d=128),
                            src[r * RS:, :],
                            eidx_sb[:, (chunk_base[gi] + cbase) * 8:
                                    (chunk_base[gi] + cbase + nch) * 8],
                            nch * 128, nidx, 128,
                            single_packet=False, queue_num=q,
                        )
                    ob8 = op8.tile([128, maxgo], u8, tag="ob8",
                                   name=f"ob8{l}_{gi}")
                    nc.scalar.dma_start(
                        out=ob8[:, :grp_osize[gi]],
                        in_=odram[:, grp_obase[gi]:
                                  grp_obase[gi] + grp_osize[gi]])
                    ob = op_.tile([128, maxgo], f16, tag="ob",
                                  name=f"ob{l}_{gi}")
                    nc.vector.tensor_copy(ob[:, :grp_osize[gi]],
                                          ob8[:, :grp_osize[gi]])

                    for i in range(wa, wb):
                        d0, w = windows[i]
                        ps = pwp.tile([128, w], f32, tag="pw",
                                      name=f"pw{l}_{i}")
                        ents = mm_lists[gi][i]
                        for k, (cl, oloc) in enumerate(ents):
                            nc.tensor.matmul(
                                out=ps[:],
                                lhsT=gt[:, cl * 128:(cl + 1) * 128],
                                rhs=ob[:, oloc:oloc + w],
                                start=(k == 0), stop=(k == len(ents) - 1),
                            )
                        nc.scalar.activation(
                            out=ftout[:, d0:d0 + w], in_=ps[:],
                            func=AF.Copy, scale=1.0 / VSCALE)

                    # ---------- dense blocks now fully covered ----------
                    if gi + 1 < len(group_info):
                        nw = group_info[gi + 1][2][0]
                        dest_end = windows[nw][0]
                    else:
                        dest_end = SHARD
                    while (blocks_done + 512 <= dest_end
                           or (gi + 1 == len(group_info)
                               and blocks_done < SHARD)):
                        b0 = blocks_done
                        blocks_done += 512
                        w = min(512, SHARD - b0)
                        lxs = ftout[:, b0:b0 + w]
                        fin_ = ftin[:, b0:b0 + w]
                        pre1 = sbp.tile([128, 512], f16, tag="pre1")
                        nc.vector.tensor_tensor(out=pre1[:, :w], in0=lxs,
                                                in1=fin_, op=add)
                        pre2 = sbp.tile([128, 512], f16, tag="pre2")
                        nc.vector.tensor_tensor(out=pre2[:, :w], in0=lxs,
                                                in1=fin_, op=mult)
                        y = pyp.tile([128, 512], f32, tag="y")
                        nc.tensor.matmul(out=y[:, :w],
                                         lhsT=wlin_sb[:, l * 128:(l + 1) * 128],
                                         rhs=pre1[:, :w], start=True, stop=False)
                        nc.tensor.matmul(out=y[:, :w],
                                         lhsT=wint_sb[:, l * 128:(l + 1) * 128],
                                         rhs=pre2[:, :w], start=False, stop=True)
                        ya = sbp.tile([128, 512], f16, tag="ya")
                        nc.scalar.activation(out=ya[:, :w], in_=y[:, :w],
                                             func=AF.Lrelu,
                                             bias=bias_sb[:, l:l + 1], scale=1.0,
                                             alpha=SLOPE)
                        sq = sbp.tile([128, 512], f16, tag="sq")
                        nc.scalar.activation(out=sq[:, :w], in_=ya[:, :w],
                                             func=AF.Square)
                        nsq = pnp.tile([1, 512], f32, tag="nsq")
                        nc.tensor.matmul(out=nsq[:, :w], lhsT=ones_c[:],
                                         rhs=sq[:, :w], start=True, stop=True)
                        rt = sbp.tile([1, 512], f16, tag="rt")
                        with nc.allow_low_precision(reason="fp16 norm"):
                            nc.scalar.activation(out=rt[:, :w], in_=nsq[:, :w],
                                                 func=AF.Abs_reciprocal_sqrt)
                        bc = pbc.tile([128, 512], f32, tag="bc")
                        nc.tensor.matmul(out=bc[:, :w], lhsT=onesrow[:],
                                         rhs=rt[:, :w], start=True, stop=True)
                        nc.vector.tensor_tensor(out=ftout[:, b0:b0 + w],
                                                in0=ya[:, :w], in1=bc[:, :w],
                                                op=mult)

                        # transpose + write out this block's rows
                        for h in range(b0, min(b0 + 512, 12544), 128):
                            nr = min(128, SHARD - h)
                            if nr <= 0:
                                break
                            tp = pwp.tile([128, 128], f16, tag="pw",
                                          name=f"tp{l}_{h}")
                            nc.tensor.transpose(out=tp[:],
                                                in_=ftout[:, h:h + 128],
                                                identity=ident[:])
                            cpo = sbp.tile([128, 128], f16, tag="cpo")
                            nc.scalar.activation(out=cpo[:], in_=tp[:],
                                                 func=AF.Copy)
                            nc.sync.dma_start(out=fshard[h:h + nr, :],
                                              in_=cpo[:nr, :])

                emit_final_level(l, feat16 if l == 0 else ags[l - 1])
                nc.gpsimd.collective_compute(
                    "AllGather", mybir.AluOpType.bypass,
                    replica_groups=[list(range(NCORE))],
                    ins=[fshard.opt()], outs=[ags[l].opt()],
                )

            emit_final_level(NL, ags[NL - 1])
            nc.sync.dma_start(out=score[:], in_=acc_t[:])

    nc.compile()
    return nc


def _meta_key(meta):
    st, NFB, fin_bounds = meta
    return (st["windows"], st["groups"], st["group_info"],
            st["chunk_base"], st["OW"], st["NCH"], NFB, fin_bounds)


def kernel(**inputs) -> np.ndarray:
    meta, in_maps, inv_perm = _pack_inputs(**inputs)
    key = _meta_key(meta)
    if key not in _cache:
        _cache[key] = _build(meta)
    nc = _cache[key]
    res = run_bass_kernel_spmd(nc, in_maps, list(range(NCORE)))
    out = np.empty(BATCH, dtype=np.float32)
    NFB = meta[1]
    for c in range(NCORE):
        sc = res.results[c]["score"]
        vals_j = sc[np.arange(NFB * 128) % 128, np.arange(NFB * 128) // 128]
        valid = inv_perm[c] >= 0
        out[inv_perm[c][valid]] = vals_j[valid]
    return out



# revision 41
# speedup vs baseline: 1.0208x; 1.0208x over previous
"""GCF message passing on 8 trn2 cores — merged-cell windowed-SpMM.

Per core (dest-shard of 12500 nodes), per layer:
  SpMM: edges sorted by (group, src-range, dest); chunks of 128 edge
        slots packed contiguously per (group, range) cell — a chunk may
        span window boundaries; each window's matmul selects its own O
        columns (zero rows for foreign slots). O is uint8 (val*2550),
        streamed via the ACT HWDGE queue and cast to fp16 on DVE; the
        1/2550 dequant rides the PSUM->SBUF copy scale. Gathered source
        rows (fp16, 256B) come from HBM dma_gather with exact
        num_idxs_reg (trailing -1 idx trimmed by the Q7 ucode) — the
        kernel is SWDGE-generation-bound at ~2.5ns/idx aggregate.
  Dense: y^T = Wlin@(Lx+F)^T + Wint@(Lx*F)^T per 512-block, fused
        bias+leaky-relu on ACT, row norm via ones-matmul +
        Abs_reciprocal_sqrt (ACT) + broadcast-matmul.
  Share: PE-transpose shard -> fshard (ping-pong) -> AllGather ags[l];
        final-dot level l is computed before collective l fires.
Final: per concat level gather u/i rows, multiply + reduce, accumulate.
"""

import os

import numpy as np

import concourse.bacc as bacc
import concourse.mybir as mybir
import concourse.tile as tile
from concourse.bass_utils import run_bass_kernel_spmd
from concourse.masks import make_identity

NUM_USERS = 30000
NUM_ITEMS = 70000
N = 100000
D = 128
NL = 3
BATCH = 16384
NCORE = 8
SHARD = N // NCORE
RS = 25000
NR = 4
WCAP = 128
KMAX = 3
GM = 48                # target chunks per gather group
VSCALE = 2550.0        # uint8 quantization scale for edge vals
BSH = BATCH // NCORE   # 2048
EPS = 1e-12
SLOPE = 0.01

f32 = mybir.dt.float32
f16 = mybir.dt.float16
i16 = mybir.dt.int16
u8 = mybir.dt.uint8

NQUEUES = int(os.environ.get("KQ", "4"))

_cache = {}


# ---------------------------------------------------------------- host side
def _build_structure(rows, cols):
    core = rows // SHARD
    dloc = rows - core * SHARD
    rng = cols // RS

    counts = np.zeros((NCORE, SHARD, NR), np.int32)
    np.add.at(counts, (core, dloc, rng), 1)

    windows = []
    cum = np.zeros((NCORE, NR), np.int64)
    d0 = 0
    for d in range(SHARD):
        c = counts[:, d, :]
        if (cum + c > 128 * KMAX).any() or d - d0 >= WCAP:
            windows.append((d0, d - d0))
            d0 = d
            cum = c.astype(np.int64).copy()
        else:
            cum += c
    windows.append((d0, SHARD - d0))
    nwin = len(windows)

    # per (window, range, core) edge counts
    wsum = np.zeros((nwin, NR, NCORE), np.int64)
    for i, (a, w) in enumerate(windows):
        wsum[i] = counts[:, a:a + w, :].sum(axis=1).T

    # group formation on estimated chunk counts
    est = np.ceil(wsum.max(axis=2) / 128).astype(np.int64).sum(axis=1)
    groups = []
    gstart, cnt = 0, 0
    for i in range(nwin):
        c = int(est[i])
        if cnt + c > GM and cnt > 0:
            groups.append((gstart, i))
            gstart, cnt = i, 0
        cnt += c
    groups.append((gstart, nwin))
    NG = len(groups)

    # merged cells: chunks packed contiguously per (group, range);
    # a chunk may span window boundaries. Shared (max-over-core) sizing.
    group_info = []     # (ncols, rlist, (wa, wb)); rlist=(r,cbase,nch,nidx)
    mm_lists = []       # per group: {win: [(ci_local, oloc)]}, ordered
    o_off_cursor = 0
    grp_obase, grp_osize = [], []
    chunk_base = [0]
    omap = {}           # (win, global_chunk) -> oloc
    for gi, (wa, wb) in enumerate(groups):
        col = 0
        rlist = []
        mm = {i: [] for i in range(wa, wb)}
        gob = o_off_cursor
        for r in range(NR):
            cellcnt = wsum[wa:wb, r, :].sum(axis=0)   # per core
            mx = int(cellcnt.max())
            if mx == 0:
                continue
            nidx = -(-mx // 16) * 16
            nch = -(-nidx // 128)
            # per-core prefix of window starts within the cell
            pre = np.zeros((wb - wa + 1, NCORE), np.int64)
            np.cumsum(wsum[wa:wb, r, :], axis=0, out=pre[1:])
            for iw in range(wa, wb):
                s0 = pre[iw - wa]
                s1 = pre[iw - wa + 1]
                act = s1 > s0
                if not act.any():
                    continue
                c_lo = int((s0[act] // 128).min())
                c_hi = int(((s1[act] - 1) // 128).max())
                for cl in range(c_lo, c_hi + 1):
                    mm[iw].append((col + cl, r))
            rlist.append((r, col, nch, nidx))
            col += nch
        # O layout + guard: every window needs >= 1 mm entry
        mm_seq = {}
        for iw in range(wa, wb):
            if not mm[iw]:
                mm[iw].append((0, -1))
            ents = []
            w = windows[iw][1]
            for (cl, r) in mm[iw]:
                oloc = o_off_cursor - gob
                omap[(iw, chunk_base[-1] + cl)] = o_off_cursor
                o_off_cursor += w
                ents.append((cl, oloc))
            mm_seq[iw] = tuple(ents)
        group_info.append((col, tuple(rlist), (wa, wb)))
        mm_lists.append(mm_seq)
        grp_obase.append(gob)
        grp_osize.append(o_off_cursor - gob)
        chunk_base.append(chunk_base[-1] + col)

    NCH = chunk_base[-1]
    g_of_win = np.zeros(nwin, np.int64)
    for gi, (wa, wb) in enumerate(groups):
        g_of_win[wa:wb] = gi
    return dict(windows=tuple(windows), groups=tuple(groups),
                group_info=tuple(group_info),
                mm_lists=tuple(mm_lists),
                grp_obase=tuple(grp_obase), grp_osize=tuple(grp_osize),
                chunk_base=tuple(chunk_base), g_of_win=g_of_win,
                omap=omap, OW=int(o_off_cursor), NCH=NCH)


def _pack_edges(rows, cols, vals, st):
    core = rows // SHARD
    dloc = rows - core * SHARD
    rng = cols // RS
    windows = st["windows"]
    nwin = len(windows)
    NCH = st["NCH"]
    NG = len(st["groups"])

    win_of = np.zeros(SHARD, np.int64)
    d0s = np.zeros(nwin, np.int64)
    for i, (a, w) in enumerate(windows):
        win_of[a:a + w] = i
        d0s[i] = a
    win_e = win_of[dloc]
    g_e = st["g_of_win"][win_e]

    # sort edges by (core, group, range, dest) -> contiguous cell streams
    key = (((core * NG + g_e) * NR + rng) * SHARD + dloc)
    order = np.argsort(key, kind="stable")
    cellkey = (core[order] * NG + g_e[order]) * NR + rng[order]
    uniq, inv, cnt = np.unique(cellkey, return_inverse=True,
                               return_counts=True)
    starts = np.zeros(len(uniq), np.int64)
    np.cumsum(cnt[:-1], out=starts[1:])
    pos = np.arange(len(cellkey)) - starts[inv]

    core_s = core[order]
    g_s = g_e[order]
    r_s = rng[order]
    win_s = win_e[order]

    # cell cbase lookup: (g, r) -> cbase
    cb = np.full((NG, NR), -1, np.int64)
    for gi, (ncols, rlist, _) in enumerate(st["group_info"]):
        for (r, cbase, nch, nidx) in rlist:
            cb[gi, r] = cbase
    ci_e = np.asarray(st["chunk_base"])[g_s] + cb[g_s, r_s] + pos // 128
    p_e = pos % 128

    eidx = np.zeros((NCORE, 16, NCH * 8), np.int16)
    col_local = (cols[order] - r_s * RS).astype(np.int16)
    eidx[core_s, p_e % 16, ci_e * 8 + p_e // 16] = col_local
    # shared trailing trim: mark [nidx, nch*128) positions as -1
    for gi, (ncols, rlist, _) in enumerate(st["group_info"]):
        for (r, cbase, nch, nidx) in rlist:
            for q in range(nidx, nch * 128):
                ci = st["chunk_base"][gi] + cbase + q // 128
                eidx[:, q % 16, ci * 8 + (q % 128) // 16] = -1

    # O fill via omap lookup table
    omap_arr = np.full((nwin, NCH), -1, np.int64)
    for (iw, ci), oloc in st["omap"].items():
        omap_arr[iw, ci] = oloc
    obase_e = omap_arr[win_s, ci_e]
    assert (obase_e >= 0).all()
    O = np.zeros((NCORE, 128, st["OW"]), np.uint8)
    j_e = dloc[order] - d0s[win_s]
    q = np.clip(np.round(vals[order] * VSCALE), 0, 255).astype(np.uint8)
    O[core_s, p_e, obase_e + j_e] = q
    return eidx, O


def _pack_final(userIdx, itemIdx):
    irow = itemIdx + NUM_USERS
    ir = irow // RS
    nfb_counts = np.zeros((NCORE, NR), np.int64)
    perms = []
    for c in range(NCORE):
        sl = slice(c * BSH, (c + 1) * BSH)
        o = np.argsort(ir[sl], kind="stable")
        perms.append(o)
        nfb_counts[c] = np.bincount(ir[sl][o], minlength=NR)
    bucket_chunks = np.ceil(nfb_counts.max(axis=0) / 128).astype(np.int64)
    fin_bounds = []
    c0 = 0
    for r in range(NR):
        n = int(bucket_chunks[r])
        if n == 0:
            continue
        fin_bounds.append((r, c0, n))
        c0 += n
    NFB = c0

    uidx_arr = np.zeros((NCORE, 16, NFB * 8), np.int16)
    iidx_arr = np.zeros((NCORE, 16, NFB * 8), np.int16)
    inv_perm = np.full((NCORE, NFB * 128), -1, np.int64)
    for c in range(NCORE):
        sl = slice(c * BSH, (c + 1) * BSH)
        o = perms[c]
        u_s = userIdx[sl][o]
        i_s = irow[sl][o]
        r_s = ir[sl][o]
        jpos = np.zeros(BSH, np.int64)
        for (r, b0, nchk) in fin_bounds:
            m = r_s == r
            jpos[m] = b0 * 128 + np.arange(int(m.sum()))
        uidx_arr[c, jpos % 16, (jpos // 128) * 8 + (jpos % 128) // 16] = \
            u_s.astype(np.int16)
        iidx_arr[c, jpos % 16, (jpos // 128) * 8 + (jpos % 128) // 16] = \
            (i_s - r_s * RS).astype(np.int16)
        inv_perm[c, jpos] = np.arange(c * BSH, (c + 1) * BSH)[o]
    return NFB, tuple(fin_bounds), uidx_arr, iidx_arr, inv_perm


def _pack_inputs(userIdx, itemIdx, rows, cols, vals, uEmbd, iEmbd,
                 Wlin, blin, Wint, bint):
    rows = np.asarray(rows, dtype=np.int64)
    cols = np.asarray(cols, dtype=np.int64)
    vals = np.asarray(vals, dtype=np.float32)
    userIdx = np.asarray(userIdx, dtype=np.int64)
    itemIdx = np.asarray(itemIdx, dtype=np.int64)

    feat0 = np.concatenate([np.asarray(uEmbd, np.float32),
                            np.asarray(iEmbd, np.float32)], axis=0)
    feat16 = np.ascontiguousarray(feat0.astype(np.float16))

    st = _build_structure(rows, cols)
    eidx, O = _pack_edges(rows, cols, vals, st)
    NFB, fin_bounds, uidx_arr, iidx_arr, inv_perm = _pack_final(
        userIdx, itemIdx)

    wlin_h = np.ascontiguousarray(
        np.asarray(Wlin, np.float32).transpose(1, 0, 2).reshape(D, NL * D)
    ).astype(np.float16)
    wint_h = np.ascontiguousarray(
        np.asarray(Wint, np.float32).transpose(1, 0, 2).reshape(D, NL * D)
    ).astype(np.float16)
    biasc = np.ascontiguousarray(
        (np.asarray(blin, np.float32) + np.asarray(bint, np.float32)).T)

    in_maps = []
    for c in range(NCORE):
        f0t = np.ascontiguousarray(
            feat16[c * SHARD:(c + 1) * SHARD].T)
        in_maps.append({
            "feat16": feat16,
            "f0t": f0t,
            "eidx": np.ascontiguousarray(np.tile(eidx[c], (8, 1))),
            "odram": np.ascontiguousarray(O[c]),
            "wlin": wlin_h,
            "wint": wint_h,
            "biasc": biasc,
            "uidx": np.ascontiguousarray(np.tile(uidx_arr[c], (8, 1))),
            "iidx": np.ascontiguousarray(np.tile(iidx_arr[c], (8, 1))),
        })

    meta = (st, NFB, fin_bounds)
    return meta, in_maps, inv_perm


# ---------------------------------------------------------------- kernel
def _build(meta):
    st, NFB, fin_bounds = meta
    windows = st["windows"]
    group_info = st["group_info"]
    mm_lists = st["mm_lists"]
    chunk_base = st["chunk_base"]
    grp_obase = st["grp_obase"]
    grp_osize = st["grp_osize"]
    NCH = st["NCH"]
    OW = st["OW"]
    nwin = len(windows)

    maxgc = max(g[0] for g in group_info)
    maxgo = max(grp_osize)

    nc = bacc.Bacc(num_devices=NCORE, num_swdge_queues=max(NQUEUES, 1))

    feat16 = nc.dram_tensor("feat16", [N, D], f16, kind="ExternalInput")
    f0t = nc.dram_tensor("f0t", [D, SHARD], f16, kind="ExternalInput")
    eidx = nc.dram_tensor("eidx", [128, NCH * 8], i16, kind="ExternalInput")
    odram = nc.dram_tensor("odram", [128, OW], u8, kind="ExternalInput")
    wlin = nc.dram_tensor("wlin", [D, NL * D], f16, kind="ExternalInput")
    wint = nc.dram_tensor("wint", [D, NL * D], f16, kind="ExternalInput")
    biasc = nc.dram_tensor("biasc", [D, NL], f32, kind="ExternalInput")
    uidx = nc.dram_tensor("uidx", [128, NFB * 8], i16, kind="ExternalInput")
    iidx = nc.dram_tensor("iidx", [128, NFB * 8], i16, kind="ExternalInput")
    score = nc.dram_tensor("score", [128, NFB], f32, kind="ExternalOutput")

    add = mybir.AluOpType.add
    mult = mybir.AluOpType.mult
    maxop = mybir.AluOpType.max
    AF = mybir.ActivationFunctionType

    with tile.TileContext(nc) as tc:
        with (
            tc.tile_pool(name="const", bufs=1) as cp,
            tc.tile_pool(name="ft", bufs=1) as ftp,
            tc.tile_pool(name="g", bufs=4) as gp,
            tc.tile_pool(name="ob8", bufs=2) as op8,
            tc.tile_pool(name="ob", bufs=2) as op_,
            tc.tile_pool(name="sb", bufs=3) as sbp,
            tc.tile_pool(name="fin", bufs=2) as fp_,
            tc.tile_pool(name="pw", bufs=4, space="PSUM") as pwp,
            tc.tile_pool(name="py", bufs=2, space="PSUM") as pyp,
            tc.tile_pool(name="pn", bufs=1, space="PSUM") as pnp,
            tc.tile_pool(name="pbc", bufs=1, space="PSUM") as pbc,
            tc.tile_pool(name="dram", bufs=1, space="DRAM") as dp,
        ):
            # ---------- constants ----------
            ident = cp.tile([128, 128], f16)
            make_identity(nc, ident[:])
            ones_c = cp.tile([128, 1], f16)
            nc.vector.memset(ones_c[:], 1.0)
            onesrow = cp.tile([1, 128], f16)
            nc.vector.memset(onesrow[:], 1.0)

            wlin_sb = cp.tile([128, NL * 128], f16)
            nc.sync.dma_start(out=wlin_sb[:], in_=wlin[:])
            wint_sb = cp.tile([128, NL * 128], f16)
            nc.sync.dma_start(out=wint_sb[:], in_=wint[:])
            bias_sb = cp.tile([128, NL], f32)
            nc.sync.dma_start(out=bias_sb[:], in_=biasc[:])

            eidx_sb = cp.tile([128, NCH * 8], i16)
            nc.sync.dma_start(out=eidx_sb[:], in_=eidx[:])
            uidx_sb = cp.tile([128, NFB * 8], i16)
            nc.sync.dma_start(out=uidx_sb[:], in_=uidx[:])
            iidx_sb = cp.tile([128, NFB * 8], i16)
            nc.sync.dma_start(out=iidx_sb[:], in_=iidx[:])

            for ii in range(4):
                gtz = gp.tile([128, maxgc * 128], f16, tag="gt",
                              name=f"gtinit{ii}")
                nc.vector.memset(gtz[:], 0.0)

            FTP = 12544  # SHARD padded to x128 for PE transposes
            fta = ftp.tile([128, FTP], f16, tag="fta")
            ftb = ftp.tile([128, FTP], f16, tag="ftb")
            nc.vector.memset(fta[:, SHARD:], 0.0)
            nc.vector.memset(ftb[:, SHARD:], 0.0)
            nc.sync.dma_start(out=fta[:, :SHARD], in_=f0t[:])

            fshards = [dp.tile([SHARD, D], f16, name=f"fsh{i}", tag=f"fsh{i}")
                       for i in range(2)]
            ags = [dp.tile([N, D], f16, name=f"ag{i}", tag=f"ag{i}",
                           addr_space="Shared") for i in range(NL)]

            acc_t = cp.tile([128, NFB], f32)

            def emit_final_level(li, srcf):
                ug = fp_.tile([128, NFB * 128], f16, tag="ug",
                              name=f"ug{li}")
                nc.gpsimd.dma_gather(
                    ug[:].rearrange("p (c d) -> p c d", d=128),
                    srcf[:],
                    uidx_sb[:],
                    NFB * 128, NFB * 128, 128,
                    single_packet=False, queue_num=(2 * li) % NQUEUES,
                )
                ig = fp_.tile([128, NFB * 128], f16, tag="ig",
                              name=f"ig{li}")
                for (rr, c0, cn) in fin_bounds:
                    nc.gpsimd.dma_gather(
                        ig[:, c0 * 128:(c0 + cn) * 128].rearrange(
                            "p (c d) -> p c d", d=128),
                        srcf[rr * RS:, :],
                        iidx_sb[:, c0 * 8:(c0 + cn) * 8],
                        cn * 128, cn * 128, 128,
                        single_packet=False,
                        queue_num=(2 * li + 1 + rr) % NQUEUES,
                    )
                nc.vector.tensor_tensor(out=ug[:], in0=ug[:], in1=ig[:],
                                        op=mult)
                sc = sbp.tile([128, NFB], f32, tag="sc")
                nc.vector.tensor_reduce(
                    out=sc[:],
                    in_=ug[:].rearrange("p (c d) -> p c d", d=128),
                    axis=mybir.AxisListType.X, op=add)
                if li == 0:
                    nc.vector.tensor_copy(acc_t[:], sc[:])
                else:
                    nc.vector.tensor_tensor(out=acc_t[:], in0=acc_t[:],
                                            in1=sc[:], op=add)

            for l in range(NL):
                fshard = fshards[l % 2]
                ftin = fta if l % 2 == 0 else ftb
                ftout = ftb if l % 2 == 0 else fta
                src = feat16 if l == 0 else ags[l - 1]

                # ---------- SpMM ----------
                blocks_done = 0
                for gi, (ncols, rlist, (wa, wb)) in enumerate(group_info):
                    gt = gp.tile([128, maxgc * 128], f16, tag="gt",
                                 name=f"gt{l}_{gi}")
                    for (r, cbase, nch, nidx) in rlist:
                        q = (r % NQUEUES) if NQUEUES > 1 else 0
                        nc.gpsimd.dma_gather(
                            gt[:, cbase * 128:(cbase + nch) * 128].rearrange(
                                "p (c d) -> p c d", d=128),
                            src[r * RS:, :],
                            eidx_sb[:, (chunk_base[gi] + cbase) * 8:
                                    (chunk_base[gi] + cbase + nch) * 8],
                            nch * 128, nidx, 128,
                            single_packet=False, queue_num=q,
                        )
                    ob8 = op8.tile([128, maxgo], u8, tag="ob8",
                                   name=f"ob8{l}_{gi}")
                    nc.scalar.dma_start(
                        out=ob8[:, :grp_osize[gi]],
                        in_=odram[:, grp_obase[gi]:
                                  grp_obase[gi] + grp_osize[gi]])
                    ob = op_.tile([128, maxgo], f16, tag="ob",
                                  name=f"ob{l}_{gi}")
                    nc.vector.tensor_copy(ob[:, :grp_osize[gi]],
                                          ob8[:, :grp_osize[gi]])

                    for i in range(wa, wb):
                        d0, w = windows[i]
                        ps = pwp.tile([128, w], f32, tag="pw",
                                      name=f"pw{l}_{i}")
                        ents = mm_lists[gi][i]
                        for k, (cl, oloc) in enumerate(ents):
                            nc.tensor.matmul(
                                out=ps[:],
                                lhsT=gt[:, cl * 128:(cl + 1) * 128],
                                rhs=ob[:, oloc:oloc + w],
                                start=(k == 0), stop=(k == len(ents) - 1),
                            )
                        nc.scalar.activation(
                            out=ftout[:, d0:d0 + w], in_=ps[:],
                            func=AF.Copy, scale=1.0 / VSCALE)

                    # ---------- dense blocks now fully covered ----------
                    if gi + 1 < len(group_info):
                        nw = group_info[gi + 1][2][0]
                        dest_end = windows[nw][0]
                    else:
                        dest_end = SHARD
                    while (blocks_done + 512 <= dest_end
                           or (gi + 1 == len(group_info)
                               and blocks_done < SHARD)):
                        b0 = blocks_done
                        blocks_done += 512
                        w = min(512, SHARD - b0)
                        lxs = ftout[:, b0:b0 + w]
                        fin_ = ftin[:, b0:b0 + w]
                        pre1 = sbp.tile([128, 512], f16, tag="pre1")
                        nc.vector.tensor_tensor(out=pre1[:, :w], in0=lxs,
                                                in1=fin_, op=add)
                        pre2 = sbp.tile([128, 512], f16, tag="pre2")
                        nc.vector.tensor_tensor(out=pre2[:, :w], in0=lxs,
                                                in1=fin_, op=mult)
                        y = pyp.tile([128, 512], f32, tag="y")
                        nc.tensor.matmul(out=y[:, :w],
                                         lhsT=wlin_sb[:, l * 128:(l + 1) * 128],
                                         rhs=pre1[:, :w], start=True, stop=False)
                        nc.tensor.matmul(out=y[:, :w],
                                         lhsT=wint_sb[:, l * 128:(l + 1) * 128],
                                         rhs=pre2[:, :w], start=False, stop=True)
                        ya = sbp.tile([128, 512], f16, tag="ya")
                        nc.scalar.activation(out=ya[:, :w], in_=y[:, :w],
                                             func=AF.Lrelu,
                                             bias=bias_sb[:, l:l + 1], scale=1.0,
                                             alpha=SLOPE)
                        sq = sbp.tile([128, 512], f16, tag="sq")
                        nc.vector.tensor_tensor(out=sq[:, :w], in0=ya[:, :w],
                                                in1=ya[:, :w], op=mult)
                        nsq = pnp.tile([1, 512], f32, tag="nsq")
                        nc.tensor.matmul(out=nsq[:, :w], lhsT=ones_c[:],
                                         rhs=sq[:, :w], start=True, stop=True)
                        rt = sbp.tile([1, 512], f16, tag="rt")
                        with nc.allow_low_precision(reason="fp16 norm"):
                            nc.scalar.activation(out=rt[:, :w], in_=nsq[:, :w],
                                                 func=AF.Abs_reciprocal_sqrt)
                        bc = pbc.tile([128, 512], f32, tag="bc")
                        nc.tensor.matmul(out=bc[:, :w], lhsT=onesrow[:],
                                         rhs=rt[:, :w], start=True, stop=True)
                        nc.vector.tensor_tensor(out=ftout[:, b0:b0 + w],
                                                in0=ya[:, :w], in1=bc[:, :w],
                                                op=mult)

                        # transpose + write out this block's rows
                        for h in range(b0, min(b0 + 512, 12544), 128):
                            nr = min(128, SHARD - h)
                            if nr <= 0:
                                break
                            tp = pwp.tile([128, 128], f16, tag="pw",
                                          name=f"tp{l}_{h}")
                            nc.tensor.transpose(out=tp[:],
                                                in_=ftout[:, h:h + 128],
                                                identity=ident[:])
                            cpo = sbp.tile([128, 128], f16, tag="cpo")
                            nc.scalar.activation(out=cpo[:], in_=tp[:],
                                                 func=AF.Copy)
                            nc.sync.dma_start(out=fshard[h:h + nr, :],
                                              in_=cpo[:nr, :])

                emit_final_level(l, feat16 if l == 0 else ags[l - 1])
                nc.gpsimd.collective_compute(
                    "AllGather", mybir.AluOpType.bypass,
                    replica_groups=[list(range(NCORE))],
                    ins=[fshard.opt()], outs=[ags[l].opt()],
                )

            emit_final_level(NL, ags[NL - 1])
            nc.sync.dma_start(out=score[:], in_=acc_t[:])

    nc.compile()
    return nc


def _meta_key(meta):
    st, NFB, fin_bounds = meta
    return (st["windows"], st["groups"], st["group_info"],
            st["chunk_base"], st["OW"], st["NCH"], NFB, fin_bounds)


def kernel(**inputs) -> np.ndarray:
    meta, in_maps, inv_perm = _pack_inputs(**inputs)
    key = _meta_key(meta)
    if key not in _cache:
        _cache[key] = _build(meta)
    nc = _cache[key]
    res = run_bass_kernel_spmd(nc, in_maps, list(range(NCORE)))
    out = np.empty(BATCH, dtype=np.float32)
    NFB = meta[1]
    for c in range(NCORE):
        sc = res.results[c]["score"]
        vals_j = sc[np.arange(NFB * 128) % 128, np.arange(NFB * 128) // 128]
        valid = inv_perm[c] >= 0
        out[inv_perm[c][valid]] = vals_j[valid]
    return out

